# revision 32
# baseline (speedup 1.0000x reference)
"""Causal self-attention (B=2, S=4096, D=512, H=8) on 8 Trainium2 NeuronCores.

Sharding: tensor-parallel over heads. Core h computes head h for both batch
elements: QKV projections for its head, causal flash attention, and its
partial (unnormalized) o_proj contribution y_h = U_h @ Wo[h*64:(h+1)*64, :]
plus the per-query softmax denominators L_h. The host computes
sum_h(y_h / L_h) + bo.

Layout (hd = 64, S = 4096, 32 k-tiles of 128 per batch):
  - All 16 xt tiles [128, 4, 512] (bf16, host pre-tiled so each block is
    one contiguous 512 KiB read) live in SBUF, prefetched on Sync.
  - Projections per 512-token block: Wq -> q_ps[0:64], [Wk|Wv] -> kv_ps
    (K.T rows 0:64, V.T rows 64:128). Bias-adds write bf16 Q.T/K.T straight
    to partition-base-0 tiles (no duplication / zero-padding DMAs); V.T is
    PE-transposed (identity at base 64) into V-natural blocks vp=[V|ones].
  - Scores: K=64 bf16 matmuls lhsT=K.T[64,128], rhs=Q.T[64,512] -> st psum
    [128, 2, 512]; one ACT exp (scale) -> P.T bf16; diagonal chunks get a
    0/1 causal mask multiply on DVE. Diagonal k-tile dj only serves query
    columns >= 128*dj, so its scores/exp/AV are column-trimmed.
    The two k-tiles of a chunk run CONCURRENTLY in the two halves of the
    PE array (row tiling: K=64 occupies only rows 0:64, so the odd k-tile
    runs at tile_position (64,0) with its own psum bank; its LDWEIGHTS
    also overlaps the even tile's matmul). This needs Q.T/K.T duplicated
    at partitions 64:128 — qt/kt are [128, S] with the upper half filled
    by sbuf->sbuf DMAs on the otherwise-idle GpSimd hwdge queue.
  - AV (bf16): U'[65, 512] += V'_kt.T @ P.T_kt; row 64 accumulates L.
  - o_proj: K=65 bf16 matmuls lhsT=U'[65,128], rhs=Wo_h' [65,512]
    (row 64 zeroed) -> y [128, 512] fp32, cast bf16, one batched DMA per
    block. U' cols 0:256 drain early (the last AV pair only touches >=256)
    so half the o_proj overlaps the seam.
  - A short warm-up spin of scratch matmuls runs during the input-DMA
    preamble so the PE_HAM clock gate releases (K=8/8, 2.4 GHz) before
    the first real matmul instead of ~13us into the run.
  - Engine split: PE all matmuls; ACT the exp stream only; DVE bias-adds,
    masks, U'/y psum->sbuf drains; Sync all DMAs (GpSimd only zeroes the
    pt pool). Emission is software-pipelined: the next block's projection
    matmuls + Q add are issued two chunks before the seam, and its first
    score chunk is woven between the current block's last exp and final AV
    pair so the PE never drains (p-state stays hot).

NOTE on correctness: the tile framework derives semaphores from emission
order — a read emitted before its writer silently reads stale SBUF. At
batch starts the first chunk consumes its own block's K/V, so proj_add_vk
and proj_vp must be emitted before that chunk (vk_done path below).
"""

import sys

for _p in ("/opt/trn_rl_repo", "/root/.axon_site/_ro/trn_rl_repo"):
    if _p not in sys.path:
        sys.path.insert(0, _p)

import numpy as np

import concourse.bass as bass
import concourse.mybir as mybir
import concourse.tile as tile
from concourse import bacc
from concourse.bass_utils import run_bass_kernel_spmd

B = 2
S = 4096
D = 512
H = 8
HD = 64
TOK = B * S          # 8192
NKT = S // 128       # 32 k-tiles per batch
NBLK = 16            # 512-token blocks over both batches
SCALE = HD ** -0.5

F32 = mybir.dt.float32
F32R = mybir.dt.float32r
BF16 = mybir.dt.bfloat16

_CACHE = {}


def _build():
    nc = bacc.Bacc("TRN2", target_bir_lowering=False, debug=False, num_devices=8)

    # xt pre-tiled on host: [NBLK*128, 4, 512] so each block's DMA is one
    # fully contiguous 512 KiB read (4 KiB per partition line).
    xt_d = nc.dram_tensor("xt", [NBLK * 128, 4, 512], BF16, kind="ExternalInput")
    # wpackA: wq [4*64] | wkv [4*128] | bq [1] | bkv [1]  (needed first)
    # wpackB: mask [4*512] | ident [64] | ones [NKT]      (needed later)
    WPA = 256 + 512 + 1 + 1
    WPB = 2048 + 64 + NKT
    wpa_d = nc.dram_tensor("wpa", [128, WPA], BF16, kind="ExternalInput")
    wpb_d = nc.dram_tensor("wpb", [128, WPB], BF16, kind="ExternalInput")
    # wo: rows 0:64 = Wo_h, rows 64:128 = the same again (host-side dup) so
    # o_proj matmul pairs can run concurrently in the two array halves.
    wo_d = nc.dram_tensor("wo", [128, D], BF16, kind="ExternalInput")
    y_d = nc.dram_tensor("y", [TOK, D], BF16, kind="ExternalOutput")
    l_d = nc.dram_tensor("l", [TOK], BF16, kind="ExternalOutput")

    blocks = [(b, tb) for b in range(B) for tb in range(8)]

    with tile.TileContext(nc) as tc:
        import contextlib

        with contextlib.ExitStack() as ctx:
            singles = ctx.enter_context(tc.tile_pool(name="singles", bufs=1))
            ptpool = ctx.enter_context(tc.tile_pool(name="pt", bufs=4))
            upool = ctx.enter_context(tc.tile_pool(name="usb", bufs=2))
            ypool = ctx.enter_context(tc.tile_pool(name="ysb", bufs=4))
            vtpool = ctx.enter_context(tc.tile_pool(name="vt", bufs=2))

            ps_st = ctx.enter_context(
                tc.tile_pool(name="ps_st", bufs=2, space="PSUM")
            )
            ps_u = ctx.enter_context(tc.tile_pool(name="ps_u", bufs=1, space="PSUM"))
            ps_misc = ctx.enter_context(
                tc.tile_pool(name="ps_misc", bufs=3, space="PSUM")
            )

            # --- constants / weights (issue order = load order on Sync) ---
            wpa_sb = singles.tile([128, WPA], BF16)
            nc.sync.dma_start(out=wpa_sb, in_=wpa_d.ap())
            wq_sb = wpa_sb[:, 0:256].rearrange("p (c m) -> p c m", c=4)
            wkv_sb = wpa_sb[:, 256:768].rearrange("p (c m) -> p c m", c=4)
            bias_sb = singles.tile([128, 2], F32)
            nc.vector.tensor_copy(bias_sb, wpa_sb[:, 768:770])
            bq_sb = bias_sb[:, 0:1]
            bkv_sb = bias_sb[:, 1:2]

            # xt0 + late-needed constants ride the idle ACT hwdge queue so
            # they overlap the weight pack on Sync during warmup.
            xts = [
                singles.tile([128, 4, 512], BF16, tag=f"xt{i}", name=f"xt{i}")
                for i in range(NBLK)
            ]
            nc.sync.dma_start(out=xts[0], in_=xt_d.ap()[0:128, :, :])

            wpb_sb = singles.tile([128, WPB], BF16)
            nc.scalar.dma_start(out=wpb_sb, in_=wpb_d.ap())
            mask_sb = wpb_sb[:, 0:2048].rearrange("p (d m) -> p d m", d=4)
            identb = wpb_sb[:, 2048:2112]
            onescol = wpb_sb[:, 2112 : 2112 + NKT]

            wo_sb = singles.tile([128, D], BF16)
            nc.scalar.dma_start(out=wo_sb, in_=wo_d.ap())
            for i in range(1, 6):
                nc.sync.dma_start(
                    out=xts[i], in_=xt_d.ap()[i * 128 : (i + 1) * 128, :, :]
                )

            def prefetch_rest():
                for i in range(6, NBLK):
                    nc.sync.dma_start(
                        out=xts[i], in_=xt_d.ap()[i * 128 : (i + 1) * 128, :, :]
                    )

            # --- persistent per-batch activation buffers ---------------
            # qt/kt: rows 0:64 = Q.T/K.T, rows 64:128 = a duplicate (DMA'd)
            # so the odd k-tile's score matmul can run in array rows 64:128
            # concurrently with the even k-tile's in rows 0:64.
            qt = [
                singles.tile([128, S], BF16, tag=f"qt_{b}", name=f"qt_{b}")
                for b in range(B)
            ]
            kt = [
                singles.tile([128, S], BF16, tag=f"kt_{b}", name=f"kt_{b}")
                for b in range(B)
            ]
            vp = [
                singles.tile([128, NKT * 65], BF16, tag=f"vp_{b}", name=f"vp_{b}")
                for b in range(B)
            ]
            for b in range(B):
                nc.vector.tensor_copy(
                    vp[b].rearrange("p (t c) -> p t c", c=65)[:, :, 64:65],
                    onescol.rearrange("p (t c) -> p t c", c=1),
                )

            def proj_mm(i):
                """Projection matmuls (PE only) for block i; kv first so the
                V transposes and K add unblock before the q group retires."""
                b, tb = blocks[i]
                xt_sb = xts[i]
                kv_ps = ps_misc.tile([128, 512], F32, tag="m")
                for c in range(4):
                    nc.tensor.matmul(
                        kv_ps,
                        wkv_sb[:, c, :],
                        xt_sb[:, c, :],
                        start=(c == 0),
                        stop=(c == 3),
                    )
                q_ps = ps_misc.tile([128, 512], F32, tag="m")
                for c in range(4):
                    nc.tensor.matmul(
                        q_ps[0:HD, :],
                        wq_sb[:, c, :],
                        xt_sb[:, c, :],
                        start=(c == 0),
                        stop=(c == 3),
                    )
                return q_ps, kv_ps

            def proj_add_q(i, q_ps):
                """Q bias-add (DVE) — unblocks the next block's first ST —
                plus the upper-half duplicate (GpSimd hwdge DMA)."""
                b, tb = blocks[i]
                cols = slice(tb * 512, (tb + 1) * 512)
                nc.vector.tensor_scalar_add(
                    qt[b][0:HD, cols], q_ps[0:HD, :], bq_sb[0:HD, 0:1]
                )
                nc.gpsimd.dma_start(
                    out=qt[b][HD:128, cols], in_=qt[b][0:HD, cols]
                )

            def proj_add_vk(i, kv_ps):
                """V bias-add + K bias-add (DVE) + K dup (GpSimd DMA)."""
                b, tb = blocks[i]
                cols = slice(tb * 512, (tb + 1) * 512)
                vt_sb = vtpool.tile([128, 512], BF16, tag="vt")
                nc.vector.tensor_scalar_add(
                    vt_sb[64:128, :], kv_ps[64:128, :], bkv_sb[64:128, 0:1]
                )
                nc.vector.tensor_scalar_add(
                    kt[b][0:HD, cols], kv_ps[0:HD, :], bkv_sb[0:HD, 0:1]
                )
                nc.gpsimd.dma_start(
                    out=kt[b][HD:128, cols], in_=kt[b][0:HD, cols]
                )
                return vt_sb

            def proj_vp(i, vt_sb):
                """V transposes (PE) + vp copies (DVE)."""
                b, tb = blocks[i]
                for j in range(4):
                    ktile = tb * 4 + j
                    vtr_ps = ps_misc.tile([128, HD], BF16, tag="m")
                    nc.tensor.transpose(
                        vtr_ps,
                        vt_sb[64:128, j * 128 : (j + 1) * 128],
                        identb[64:128, :],
                    )
                    nc.vector.tensor_copy(
                        vp[b][:, ktile * 65 : ktile * 65 + 64], vtr_ps
                    )

            class AttnState:
                """Carried emission state for one q-block's attention."""

                def __init__(self, i):
                    self.b, self.qb = blocks[i]
                    self.q0 = self.qb * 512
                    self.n_chunks = 2 * (self.qb + 1)
                    self.u_ps = ps_u.tile([128, 512], F32, tag="u")
                    self.prev_pt = None
                    self.pt_prev2 = None
                    self.j = 0
                    self.spread = []  # deferred PE work woven between chunks

                def c0(self, ktile):
                    dj = ktile - 4 * self.qb
                    return 128 * dj if dj > 0 else 0

                def emit_av(self, pt, j):
                    for j2 in range(2):
                        ktile = 2 * j + j2
                        c0 = self.c0(ktile)
                        nc.tensor.matmul(
                            self.u_ps[0:65, c0:512],
                            vp[self.b][:, ktile * 65 : ktile * 65 + 65],
                            pt[:, j2, c0:512],
                            start=(ktile == 0),
                            stop=(ktile == 2 * self.n_chunks - 1),
                            skip_group_check=True,
                        )

                def emit_pad(self):
                    """PE filler into the unused u_ps partitions 96:128:
                    keeps the PE_HAM activity monitor from re-throttling
                    the clock when real PE work runs ahead of the ACT-paced
                    exp stream. Output is never read."""
                    nc.tensor.matmul(
                        self.u_ps[96:128, 0:96],
                        spin_sb[0:64, 0:32],
                        spin_sb[0:64, 0:96],
                        start=True,
                        stop=True,
                        skip_group_check=True,
                        tile_position=(0, 96),
                    )

                def emit_chunk(self, skip_av=False):
                    """One chunk: ST pair (PE), exp (ACT), mask (DVE), and
                    the previous chunk's AV pair (PE). Diagonal k-tile dj
                    only serves queries with column >= 128*dj — scores/exp/
                    AV are trimmed to that range. skip_av defers the
                    previous AV pair so the caller can wedge independent PE
                    work into the exp window."""
                    j = self.j
                    st = ps_st.tile([128, 2, 512], F32, tag="st")
                    for j2 in range(2):
                        # j2=1 runs in array rows 64:128 (tile_position
                        # (64,0) auto-derived from the operand partition
                        # base) CONCURRENTLY with j2=0 in rows 0:64.
                        ktile = 2 * j + j2
                        c0 = self.c0(ktile)
                        r = slice(HD * j2, HD * j2 + HD)
                        nc.tensor.matmul(
                            st[:, j2, c0:512],
                            kt[self.b][r, ktile * 128 : (ktile + 1) * 128],
                            qt[self.b][r, self.q0 + c0 : self.q0 + 512],
                            start=True,
                            stop=True,
                        )
                    if self.spread:
                        self.spread.pop(0)()  # one deferred o_proj MM/cast
                    else:
                        self.emit_pad()
                    pt = ptpool.tile([128, 2, 512], BF16, tag="pt")
                    c0e = 256 if j == self.n_chunks - 1 else 0
                    nc.scalar.activation(
                        pt[:, :, c0e:512],
                        st[:, :, c0e:512],
                        mybir.ActivationFunctionType.Exp,
                        scale=SCALE,
                    )
                    if j >= self.n_chunks - 2:  # diagonal chunks: causal mask
                        d0 = (j % 2) * 2
                        nc.vector.tensor_mul(
                            pt[:, :, c0e:512],
                            pt[:, :, c0e:512],
                            mask_sb[:, d0 : d0 + 2, c0e:512],
                        )
                    if self.prev_pt is not None and not skip_av:
                        self.emit_av(self.prev_pt, j - 1)
                    self.pt_prev2 = self.prev_pt
                    self.prev_pt = pt
                    self.j += 1

            def tail_u_a(i, stt):
                """Early U' drain: cols 0:256 are final once AV(n-2) ran —
                the last AV pair only accumulates into cols >= 256."""
                u_sb = upool.tile([128, 512], BF16, tag="u")
                nc.vector.tensor_copy(u_sb[0:65, 0:256], stt.u_ps[0:65, 0:256])
                return u_sb

            def tail_u_b(i, stt, u_sb):
                """Late U' drain (DVE) + L out (Sync hwdge) + U dup into
                rows 64:128 (GpSimd DMA, ordered after the L row is read)
                so the o_proj matmul pairs can run row-tiled."""
                b, qb = blocks[i]
                nc.vector.tensor_copy(u_sb[0:65, 256:512], stt.u_ps[0:65, 256:512])
                row0 = b * S + qb * 512
                nc.sync.dma_start(
                    out=l_d.ap()[row0 : row0 + 512].rearrange("(p c) -> p c", p=1),
                    in_=u_sb[64:65, :],
                )
                nc.gpsimd.dma_start(out=u_sb[64:128, :], in_=u_sb[0:64, :])
                return row0

            def make_oproj_thunks(row0, u_sb):
                """y = U'.T @ Wo (unnormalized); K=64 bf16. Each MM thunk
                emits a row-tiled CONCURRENT pair: even j2 in array rows
                0:64, odd j2 in rows 64:128 (u/wo duplicated there).
                Thunks weave into the NEXT block's chunk stream (PE filler
                that keeps the p-state hot)."""
                y_sb = ypool.tile([128, 4, 512], BF16, tag="y")
                state = {}

                def mk_mm(j2):
                    def f():
                        ps = []
                        for jj in (j2, j2 + 1):
                            r = slice(64 * (jj % 2), 64 * (jj % 2) + 64)
                            y_ps = ps_misc.tile([128, 512], F32, tag="m")
                            nc.tensor.matmul(
                                y_ps,
                                u_sb[r, jj * 128 : (jj + 1) * 128],
                                wo_sb[r, :],
                                start=True,
                                stop=True,
                            )
                            ps.append(y_ps)
                        state[j2] = ps

                    return f

                def mk_cast(j2):
                    def f():
                        for k, y_ps in enumerate(state.pop(j2)):
                            nc.vector.tensor_copy(y_sb[:, j2 + k, :], y_ps)

                    return f

                def dma():
                    nc.sync.dma_start(
                        out=y_d.ap()[row0 : row0 + 512, :].rearrange(
                            "(j p) c -> p j c", p=128
                        ),
                        in_=y_sb,
                    )

                return [mk_mm(0), mk_cast(0), mk_mm(2), mk_cast(2), dma]

            # Zero the pt pool once: trimmed exp calls leave stale columns
            # that the causal mask multiplies by 0 — NaN*0 must not happen.
            for k in range(4):
                ptz = ptpool.tile([128, 2, 512], BF16, tag="pt")
                nc.gpsimd.memset(ptz, 0)

            # Warm-up spin: dense scratch matmuls during the input-DMA
            # preamble flip the PE_HAM clock gate to K=8/8 (~3.4us of
            # sustained activity) so real work starts at 2.4 GHz. The
            # results are never read.
            spin_sb = singles.tile([128, 128], BF16, tag="spin")
            nc.vector.memset(spin_sb, 0)
            spin_ps = ps_misc.tile([128, 128], F32, tag="m")
            for k in range(18):
                nc.tensor.matmul(
                    spin_ps,
                    spin_sb,
                    spin_sb,
                    start=True,
                    stop=True,
                    skip_group_check=True,
                )

            # Software pipeline. Loop invariant: stt is block i's attention
            # with chunk 0 already emitted. Block i+1's proj matmuls and
            # first chunk are emitted inside block i's last-exp/tail window
            # so the PE never drains (p-state stays hot).
            q_ps, kv_ps = proj_mm(0)
            proj_add_q(0, q_ps)
            proj_vp(0, proj_add_vk(0, kv_ps))
            stt = AttnState(0)
            stt.emit_chunk()
            for i in range(NBLK):
                nxt = i + 1 if i + 1 < NBLK else None
                while stt.j < stt.n_chunks - 2:
                    stt.emit_chunk()
                # pre-seam: next block's proj matmuls + Q add land two
                # chunks early so Qadd clears ACT before the seam
                if nxt is not None:
                    q_ps, kv_ps = proj_mm(nxt)
                    proj_add_q(nxt, q_ps)
                if stt.j < stt.n_chunks - 1:
                    stt.emit_chunk()
                # last chunk: defer AV(n-2); the seam weaves the next
                # block's first ST between it and the exp/mask chain
                stt.emit_chunk(skip_av=True)
                vk_done = False
                stt2 = None
                if nxt is not None:
                    if blocks[nxt][1] == 0:
                        # batch start: block nxt's first chunk reads its OWN
                        # kt/vp — their writers must be emitted first (the
                        # tile framework cannot see forward dependencies).
                        proj_vp(nxt, proj_add_vk(nxt, kv_ps))
                        vk_done = True
                    stt2 = AttnState(nxt)
                    stt2.emit_chunk()
                while stt.spread:  # leftover deferred work: extra PE runway
                    stt.spread.pop(0)()
                if stt.pt_prev2 is not None:
                    stt.emit_av(stt.pt_prev2, stt.n_chunks - 2)
                u_sb = tail_u_a(i, stt)
                stt.emit_av(stt.prev_pt, stt.n_chunks - 1)
                row0 = tail_u_b(i, stt, u_sb)
                if nxt is not None and not vk_done:
                    proj_vp(nxt, proj_add_vk(nxt, kv_ps))
                thunks = make_oproj_thunks(row0, u_sb)
                if nxt is not None:
                    stt2.spread.extend(thunks)
                else:
                    for t in thunks:
                        t()
                if i == 1:
                    prefetch_rest()
                if nxt is not None:
                    stt = stt2

    nc.compile()
    return nc


def _prep_inputs(x, Wq, bq, Wk, bk, Wv, bv, Wo, bo):
    import ml_dtypes

    bf16 = ml_dtypes.bfloat16
    # xt pre-tiled: [NBLK, 128, 4, 512] -> [NBLK*128, 4, 512] contiguous
    xt = x.reshape(TOK, D).T  # [512, 8192]
    xtt = np.empty((NBLK, 128, 4, 512), dtype=np.float32)
    for i in range(NBLK):
        xtt[i] = xt[:, i * 512 : (i + 1) * 512].reshape(4, 128, 512).transpose(
            1, 0, 2
        )
    xtt = np.ascontiguousarray(xtt.reshape(NBLK * 128, 4, 512)).astype(bf16)

    mask = np.zeros((128, 4, 512), dtype=np.float32)
    p = np.arange(128)[:, None]
    c = np.arange(512)[None, :]
    for d in range(4):
        mask[:, d, :] = (p + 128 * d <= c).astype(np.float32)
    identb = np.zeros((128, 64), dtype=np.float32)
    identb[64:128, :] = np.eye(64, dtype=np.float32)

    def pack_w(w):  # [512, M] -> [128, 4*M] in "(c p) m" tile order
        m = w.shape[1]
        return w.reshape(4, 128, m).transpose(1, 0, 2).reshape(128, 4 * m)

    in_maps = []
    for h in range(H):
        hs = slice(h * HD, (h + 1) * HD)
        wo_h = np.concatenate([Wo[hs, :], Wo[hs, :]], axis=0).astype(bf16)
        wpa = np.concatenate(
            [
                pack_w(Wq[:, hs]),                                   # 256
                pack_w(np.concatenate([Wk[:, hs], Wv[:, hs]], 1)),   # 512
                np.concatenate([bq[hs], bq[hs]]).reshape(128, 1),    # 1
                np.concatenate([bk[hs], bv[hs]]).reshape(128, 1),    # 1
            ],
            axis=1,
        ).astype(bf16)
        wpb = np.concatenate(
            [
                mask.reshape(128, 2048),                             # 2048
                identb,                                              # 64
                np.ones((128, NKT), dtype=np.float32),               # NKT
            ],
            axis=1,
        ).astype(bf16)
        in_maps.append({"xt": xtt, "wpa": wpa, "wpb": wpb, "wo": wo_h})
    return in_maps


def _install_ntff_hook():
    """Register the axon NTFF profiling hook (test-only plumbing)."""
    import types

    try:
        from antenv.axon_hooks import set_axon_ntff_profile_hook  # noqa: F401
    except ImportError:
        m = types.ModuleType("antenv.axon_hooks")
        m._HOOK = None
        m.set_axon_ntff_profile_hook = lambda h: setattr(m, "_HOOK", h)
        m.get_axon_ntff_profile_hook = lambda: m._HOOK
        sys.modules["antenv.axon_hooks"] = m
        import antenv

        antenv.axon_hooks = m
    from antenv.axon_hooks import (
        get_axon_ntff_profile_hook,
        set_axon_ntff_profile_hook,
    )

    if get_axon_ntff_profile_hook() is None:
        import trn_agent_boot.trn_boot as tb

        set_axon_ntff_profile_hook(
            tb._ntff_profile_via_ctypes("/opt/axon/libaxon_pjrt.so")
        )


def kernel(x, Wq, bq, Wk, bk, Wv, bv, Wo, bo, _trace=False):
    x, Wq, bq, Wk, bk, Wv, bv, Wo, bo = (
        np.asarray(a, dtype=np.float32) for a in (x, Wq, bq, Wk, bk, Wv, bv, Wo, bo)
    )
    if "nc" not in _CACHE:
        _CACHE["nc"] = _build()
    nc = _CACHE["nc"]
    in_maps = _prep_inputs(x, Wq, bq, Wk, bk, Wv, bv, Wo, bo)
    kwargs = {}
    if _trace:
        _install_ntff_hook()
        kwargs = dict(trace=True, trace_cores=[0])
    res = run_bass_kernel_spmd(nc, in_maps, core_ids=list(range(8)), **kwargs)
    _CACHE["last_result"] = res
    y = np.zeros((TOK, D), dtype=np.float64)
    for r in res.results:
        y += r["y"].astype(np.float64) / r["l"].astype(np.float64)[:, None]
    y += bo[None, :]
    return y.astype(np.float32).reshape(B, S, D)

